# revision 29
# baseline (speedup 1.0000x reference)
"""Single-head attention (B=8, S=2048, D_in=D_out=1024) on 8 Trainium2 NeuronCores.

Sharding: data-parallel over batch — core b computes batch element b end-to-end.
Weights (W_K/W_V/W_Q, 4 MB each) are replicated to every core.

Design (vs the float32r baseline at ~500-580us measured): all matmul operands
are fp16 (same PE rate as float32r — 1 cyc/row — at half the SBUF footprint),
everything stays SBUF-resident, and the PE does no transposes at all:

  Projections. X [s,d] is DMA'd in fp32, cast to fp16 on the otherwise-idle
  GPSIMD engine, and transposed SBUF->SBUF per 128-row tile by the DMA XBAR
  ucode transpose (dma_start_transpose, 2-byte dtypes, 14ns per 16x128 tile)
  — the 384 PE identity-matmul transposes of the baseline become DMA-engine
  work that overlaps with PE matmuls.
    K^T tile [128 e, s]  = accum_d  W[d,e-slice]^T' @ X^T[d, s-chunk]
    Q^T tile [128 e, i]  = same (per query-half)
    V  tile [128 s, e]   = accum_d  X^T[d,s-slice]^T' @ W[d, e-chunk]
  Attention, per query-half (so Q^T 16KB + P^T 32KB coexist with K^T+V 64KB
  and the staging pools):
  B1 (scores, per 128-key tile j): computed directly TRANSPOSED:
    S^T chunk [128 j, 512 i] = accum_e kt[e][:, j-slice]^T' @ qt[e][:, i-chunk]
    P^T = exp(S^T/32 - 12) on ACT (PSUM fp32 in, fp16 SBUF out). The -12
    shift keeps P inside fp16 range (scores are O(+-13) for this data);
    softmax is shift-invariant so the 1/rowsum normalization cancels it.
    Scores come out already transposed, so the baseline's 256 PE transposes
    of P vanish and P^T feeds B2 directly as the stationary operand.
  B2 (output, per 128-query tile i):
    Z [128 i, e-512]  = accum_j pt[j][:, i-slice]^T' @ vt[j][:, e-chunk]
    rowsum [128 i, 1] = accum_j pt[j][:, i-slice]^T' @ ones[128,1]
      (reuses the stationary tile the PE just loaded for the Z matmuls —
      one extra moving row, nearly free)
    z = Z * (1/rowsum) fused into the PSUM->SBUF copy (DVE), DMA out fp32.

Scheduling (found by timeline-sim gap analysis):
  - Pipeline order per repeat: K-proj, Q-h0-proj, B1-h0, V-proj, B2-h0,
    Q-h1-proj, B1-h1, B2-h1. Each stage's DMA+cast demand lands inside the
    previous stage's PE window, so the serialized DMA resource is never
    oversubscribed against the PE (phase-A-only demand ~157us exceeds the
    A window; interleaved it doesn't).
  - All input DMAs and XBAR transposes issue on the SP queue; exp and the
    z-output DMAs on the ACT queue; fp32->fp16 casts run on GPSIMD. Keeping
    casts off the ACT/DVE queues matters: the tile scheduler encodes PSUM
    slot-reuse deps as engine-stream position thresholds, so a DMA-blocked
    cast hoisted ahead of the exps stalls B1's matmuls (cost 47us/rep).
  - All staging and resident pools are created once at top level and
    tag-rotated per repeat, so with R repeats in one NEFF, repeat n+1's
    X/W prefetch runs during repeat n's attention phases and the PE never
    waits on DMA at a repeat boundary.

PE budget per core: 1792 N=512 matmuls @ ~213ns + 256 N=1 matmuls ~= 405us
of PE busy (vs ~462us for the baseline, which adds 640 PE transposes at
fp32 2cyc/row). Timeline-sim steady-state slope 410us/rep (PE 98.5% busy);
measured harness slope ~300-310us/rep (vs 502-581us baseline).

Numerics: fp16 has a 10-bit mantissa; PSUM accumulation is fp32. Measured
end-to-end relative error vs the fp32 reference is 1.08e-3 (gate: 2e-2).
"""

from contextlib import ExitStack

import numpy as np

import concourse.bacc as bacc
import concourse.mybir as mybir
import concourse.tile as tile

F32 = mybir.dt.float32
H16 = mybir.dt.float16

B, S, D = 8, 2048, 1024
P = 128                    # SBUF partitions
TS = S // P                # 16 seq tiles
TD = D // P                # 8 d/e blocks
CH = 512                   # phase-A seq quarter (matmul free dim)
NCH = S // CH              # 4 quarters
TPC = CH // P              # 4 seq tiles per quarter
HS = S // 2                # query-half size for phase B
IC = 512                   # phase-B1 query chunk (mov free dim)
NIC = HS // IC             # 2 chunks per half
EC = 512                   # phase-B2 value-dim chunk
NEC = D // EC              # 2
SCALE = 1.0 / float(np.sqrt(D))
EXP_BIAS = -12.0           # softmax shift (cancelled by the 1/rowsum scale)


def build_program(repeats: int = 1, phases: str = "ab"):
    nc = bacc.Bacc("TRN2", target_bir_lowering=False, debug=False)

    xk = nc.dram_tensor("xk", [S, D], F32, kind="ExternalInput").ap()
    xv = nc.dram_tensor("xv", [S, D], F32, kind="ExternalInput").ap()
    xq = nc.dram_tensor("xq", [S, D], F32, kind="ExternalInput").ap()
    wk = nc.dram_tensor("wk", [D, D], F32, kind="ExternalInput").ap()
    wv = nc.dram_tensor("wv", [D, D], F32, kind="ExternalInput").ap()
    wq = nc.dram_tensor("wq", [D, D], F32, kind="ExternalInput").ap()
    z = nc.dram_tensor("z", [S, D], F32, kind="ExternalOutput").ap()

    with tile.TileContext(nc) as tc, ExitStack() as ctx:
        top = ctx.enter_context(tc.tile_pool(name="top", bufs=1))
        ones = top.tile([P, 1], H16, tag="ones", name="ones")
        nc.vector.memset(ones[:], 1.0)
        ebias = top.tile([P, 1], F32, tag="ebias", name="ebias")
        nc.vector.memset(ebias[:], EXP_BIAS)

        # Persistent pools: same tags rotate across repeats, which both
        # bounds SBUF and lets repeat n+1's staging DMAs overlap repeat n's
        # phase B (no address aliasing against the B-phase pools).
        pools = {
            "res": ctx.enter_context(tc.tile_pool(name="res", bufs=1)),
            "wst": ctx.enter_context(tc.tile_pool(name="wst", bufs=2)),
            "xst": ctx.enter_context(tc.tile_pool(name="xst", bufs=1)),
            "ptp": ctx.enter_context(tc.tile_pool(name="ptp", bufs=1)),
            "zop": ctx.enter_context(tc.tile_pool(name="zop", bufs=1)),
            "scp": ctx.enter_context(tc.tile_pool(name="scp", bufs=2)),
        }

        for rep in range(repeats):
            _one_pass(nc, tc, pools, ones, ebias, xk, xv, xq, wk, wv, wq, z, rep, phases)

    nc.compile()
    return nc


def _one_pass(nc, tc, pools, ones, ebias, xk, xv, xq, wk, wv, wq, z, rep, phases="ab"):
    res, wst, xst = pools["res"], pools["wst"], pools["xst"]
    ptp, zop, scp = pools["ptp"], pools["zop"], pools["scp"]

    # fp16 residents: K^T and Q^T as 8 [128 e, 2048 s] tiles, V as 16
    # [128 s, 1024 e] tiles. 96 KB/partition total.
    # Xk^T resident directly (no K projection): [128 d_lo, 8 d_hi, 2048 j]
    xkt = res.tile([P, TD, S], H16, tag="xkt", name="xkt")
    vt = [res.tile([P, D], H16, tag=f"v{j}", name=f"v{j}") for j in range(TS)]

    # ---------------- Phase A + B, Q interleaved per half ----------------
    with tc.tile_pool(name=f"psA{rep}", bufs=3, space="PSUM") as psA:

        def stage_x_quarter(x_dram, q):
            """Load+cast+XBAR-transpose one 512-row quarter of X into a
            [128 d_lo, 8 d_hi, 512 s] fp16 tile (DMA+Pool engines only)."""
            xtq = xst.tile([P, TD, CH], H16, tag="xtq", name="xtq", bufs=4)
            for t in range(TPC):
                row = (q * TPC + t) * P
                xbf = xst.tile([P, D], H16, tag="xbf", name="xbf", bufs=3)
                for xh in range(2):
                    xf = xst.tile([P, D // 2], F32, tag="xf", name="xf", bufs=4)
                    nc.sync.dma_start(
                        xf[:],
                        x_dram[row : row + P, xh * (D // 2) : (xh + 1) * (D // 2)],
                    )
                    nc.gpsimd.tensor_copy(
                        xbf[:, xh * (D // 2) : (xh + 1) * (D // 2)], xf[:]
                    )
                nc.sync.dma_start_transpose(xtq[:, :, t * P : (t + 1) * P], xbf[:])
            return xtq

        def stage_w(w_dram):
            """Load W fp32 and cast to fp16 [128 d_lo, 8 d_hi, 1024 e]."""
            wbf = wst.tile([P, TD, D], H16, tag="wbf", name="wbf")
            for dh in range(TD):
                for wh in range(2):
                    wf = wst.tile([P, D // 2], F32, tag="wf", name="wf", bufs=2)
                    nc.sync.dma_start(
                        wf[:],
                        w_dram[
                            dh * P : (dh + 1) * P,
                            wh * (D // 2) : (wh + 1) * (D // 2),
                        ],
                    )
                    nc.gpsimd.tensor_copy(
                        wbf[:, dh, wh * (D // 2) : (wh + 1) * (D // 2)], wf[:]
                    )
            return wbf

        def stage_xk():
            # Xk fp32 -> fp16 -> XBAR straight into the resident xkt tile.
            for st_i in range(TS):
                row = st_i * P
                xbf = xst.tile([P, D], H16, tag="xbf", name="xbf", bufs=3)
                for xh in range(2):
                    xf = xst.tile([P, D // 2], F32, tag="xf", name="xf", bufs=4)
                    nc.sync.dma_start(
                        xf[:],
                        xk[row : row + P, xh * (D // 2) : (xh + 1) * (D // 2)],
                    )
                    nc.gpsimd.tensor_copy(
                        xbf[:, xh * (D // 2) : (xh + 1) * (D // 2)], xf[:]
                    )
                nc.sync.dma_start_transpose(
                    xkt[:, :, st_i * P : (st_i + 1) * P], xbf[:]
                )

        def stage_wT(wbf_src):
            # W^T [128 e_lo, 8 e_hi, 1024 d] staged as two xtq-shaped tiles
            # (d 0:512 and 512:1024), via one XBAR per 128-row d-block.
            wt = [
                xst.tile([P, TD, CH], H16, tag="xtq", name="wt", bufs=4)
                for _ in range(2)
            ]
            for dh in range(TD):
                nc.sync.dma_start_transpose(
                    wt[dh // 4][:, :, (dh % 4) * P : (dh % 4 + 1) * P],
                    wbf_src[:, dh, :],
                )
            return wt

        def build_m(wqT, wkT):
            # M[d, d'] = sum_e Wq[d,e] Wk[d',e], stored like a wbf
            # [128 d_lo, 8 d_hi, 1024 d'] so A^T-proj reuses the Q-proj form.
            m = wst.tile([P, TD, D], H16, tag="wbf", name="m")
            for dh in range(TD):
                for c in range(2):
                    ps = psA.tile([P, CH], F32, tag="proj", name="proj_ps")
                    stat_t = wqT[dh // 4]
                    for eh in range(TD):
                        nc.tensor.matmul(
                            ps[:],
                            stat_t[:, eh, (dh % 4) * P : (dh % 4 + 1) * P],
                            wkT[c][:, eh, :],
                            start=(eh == 0),
                            stop=(eh == TD - 1),
                        )
                    nc.vector.tensor_copy(m[:, dh, c * CH : (c + 1) * CH], ps[:])
            return m

        def proj_v(wbf, xtq, q):
            # V tiles [128 s, 512 e] = accum_d X^T[d,s-slice]^T' @ W[d,e]
            for t in range(TPC):
                for ec in range(NEC):
                    ps = psA.tile([P, EC], F32, tag="proj", name="proj_ps")
                    for dh in range(TD):
                        nc.tensor.matmul(
                            ps[:],
                            xtq[:, dh, t * P : (t + 1) * P],
                            wbf[:, dh, ec * EC : (ec + 1) * EC],
                            start=(dh == 0),
                            stop=(dh == TD - 1),
                        )
                    nc.vector.tensor_copy(
                        vt[q * TPC + t][:, ec * EC : (ec + 1) * EC], ps[:]
                    )

        def proj_q_half(wbf_q, half, qt):
            for ql in range(NCH // 2):
                q = half * (NCH // 2) + ql
                xtq = stage_x_quarter(xq, q)
                for e in range(TD):
                    ps = psA.tile([P, CH], F32, tag="proj", name="proj_ps")
                    for dh in range(TD):
                        nc.tensor.matmul(
                            ps[:],
                            wbf_q[:, dh, e * P : (e + 1) * P],
                            xtq[:, dh, :],
                            start=(dh == 0),
                            stop=(dh == TD - 1),
                        )
                    nc.vector.tensor_copy(
                        qt[e][:, ql * CH : (ql + 1) * CH], ps[:]
                    )

        def b1_half(qt, pt):
            with tc.tile_pool(name=f"ps1_{rep}_{id(pt)}", bufs=2, space="PSUM") as ps1:
                for j in range(TS):
                    st = [
                        ps1.tile([P, IC], F32, tag=f"st{ic}", name=f"st{ic}")
                        for ic in range(NIC)
                    ]
                    for e in range(TD):
                        stat = xkt[:, e, j * P : (j + 1) * P]
                        for ic in range(NIC):
                            nc.tensor.matmul(
                                st[ic][:],
                                stat,
                                qt[e][:, ic * IC : (ic + 1) * IC],
                                start=(e == 0),
                                stop=(e == TD - 1),
                            )
                    for ic in range(NIC):
                        # exp(s/32 - 12): the shift keeps P inside fp16
                        # range; the 1/rowsum normalization cancels it.
                        nc.scalar.activation(
                            pt[j][:, ic * IC : (ic + 1) * IC],
                            st[ic][:],
                            mybir.ActivationFunctionType.Exp,
                            scale=SCALE,
                            bias=ebias[:],
                        )

        def b2_half(half, pt):
            with tc.tile_pool(name=f"ps2_{rep}_{half}", bufs=2, space="PSUM") as ps2:
                for il in range(TS // 2):
                    it = half * (TS // 2) + il
                    zps = [
                        ps2.tile([P, EC], F32, tag=f"z{ec}", name=f"z{ec}")
                        for ec in range(NEC)
                    ]
                    sps = ps2.tile([P, 1], F32, tag="sm", name="sm", bufs=1)
                    for j in range(TS):
                        stat = pt[j][:, il * P : (il + 1) * P]
                        for ec in range(NEC):
                            nc.tensor.matmul(
                                zps[ec][:],
                                stat,
                                vt[j][:, ec * EC : (ec + 1) * EC],
                                start=(j == 0),
                                stop=(j == TS - 1),
                            )
                        nc.tensor.matmul(
                            sps[:],
                            stat,
                            ones[:],
                            start=(j == 0),
                            stop=(j == TS - 1),
                        )
                    rec = scp.tile([P, 1], F32, tag="rec", name="rec")
                    nc.vector.reciprocal(rec[:], sps[:])
                    for ec in range(NEC):
                        zo = zop.tile(
                            [P, EC], F32, tag=f"zo{ec}", name=f"zo{ec}", bufs=3
                        )
                        nc.vector.tensor_scalar_mul(zo[:], zps[ec][:], rec[:])
                        nc.scalar.dma_start(
                            z[it * P : (it + 1) * P, ec * EC : (ec + 1) * EC],
                            zo[:],
                        )

        # Pipeline order: M-build, A^T-h0, B1-h0, V, B2-h0, A^T-h1, B1-h1,
        # B2-h1, with Xk^T XBAR'd straight into residence (no K projection:
        # scores = Xq (Wq Wk^T) Xk^T, and M = Wq Wk^T costs 128 matmuls vs
        # the K projection's 256). Each stage's DMA+cast demand lands inside
        # the previous stage's PE window.
        wbf_q = stage_w(wq)
        wbf_k = stage_w(wk)
        wqT = stage_wT(wbf_q)
        wkT = stage_wT(wbf_k)
        m = build_m(wqT, wkT)
        stage_xk()

        if phases == "a":
            wbf_v = stage_w(wv)
            for q in range(NCH):
                proj_v(wbf_v, stage_x_quarter(xv, q), q)
            for it in range(TS):
                dummy = zop.tile([P, D], F32, tag="dummy", name="dummy")
                nc.vector.tensor_copy(dummy[:], vt[it][:])
                nc.scalar.dma_start(z[it * P : (it + 1) * P, :], dummy[:])
            return

        qt0 = [res.tile([P, HS], H16, tag=f"qt{e}", name=f"qt{e}") for e in range(TD)]
        proj_q_half(m, 0, qt0)
        pt0 = [ptp.tile([P, HS], H16, tag=f"pt{j}", name=f"pt{j}") for j in range(TS)]
        b1_half(qt0, pt0)

        wbf_v = stage_w(wv)
        for q in range(NCH):
            proj_v(wbf_v, stage_x_quarter(xv, q), q)

        b2_half(0, pt0)

        qt1 = [res.tile([P, HS], H16, tag=f"qt{e}", name=f"qt{e}") for e in range(TD)]
        proj_q_half(m, 1, qt1)
        pt1 = [ptp.tile([P, HS], H16, tag=f"pt{j}", name=f"pt{j}") for j in range(TS)]
        b1_half(qt1, pt1)
        b2_half(1, pt1)


_EXEC = None
_EXEC_BODY = None


def _build_exec(nc=None):
    """Compile the per-core program and wrap it in one jitted 8-core SPMD
    callable (shard_map over the 8 NeuronCores). Built once per process; the
    same callable serves correctness runs and timing loops."""
    import jax
    from jax.experimental.shard_map import shard_map
    from jax.sharding import Mesh, PartitionSpec

    from concourse import bass2jax

    if nc is None:
        nc = build_program()
    bass2jax.install_neuronx_cc_hook()

    partition_name = nc.partition_id_tensor.name if nc.partition_id_tensor else None
    in_names, out_names, out_avals, zero_outs = [], [], [], []
    for alloc in nc.m.functions[0].allocations:
        if not isinstance(alloc, mybir.MemoryLocationSet):
            continue
        name = alloc.memorylocations[0].name
        if alloc.kind == "ExternalInput":
            if name != partition_name:
                in_names.append(name)
        elif alloc.kind == "ExternalOutput":
            assert alloc.tensor_shape is not None and alloc.dtype is not None
            out_names.append(name)
            shape = tuple(alloc.tensor_shape)
            dtype = mybir.dt.np(alloc.dtype)
            out_avals.append(jax.core.ShapedArray(shape, dtype))
            zero_outs.append(np.zeros(shape, dtype))
    n_params = len(in_names)
    all_in_names = tuple(in_names) + tuple(out_names)
    if partition_name is not None:
        all_in_names = all_in_names + (partition_name,)

    def _body(*args):
        operands = list(args)
        if partition_name is not None:
            operands.append(bass2jax.partition_id_tensor())
        outs = bass2jax._bass_exec_p.bind(
            *operands,
            out_avals=tuple(out_avals),
            in_names=all_in_names,
            out_names=tuple(out_names),
            lowering_input_output_aliases=(),
            sim_require_finite=True,
            sim_require_nnan=True,
            nc=nc,
        )
        return tuple(outs)

    devices = jax.devices()[:B]
    assert len(devices) == B, f"need {B} cores, have {len(jax.devices())}"
    mesh = Mesh(np.asarray(devices), ("core",))
    n_outs = len(out_names)
    sharded_body = shard_map(
        _body,
        mesh=mesh,
        in_specs=(PartitionSpec("core"),) * (n_params + n_outs),
        out_specs=(PartitionSpec("core"),) * n_outs,
        check_rep=False,
    )
    global _EXEC_BODY
    _EXEC_BODY = sharded_body
    fn = jax.jit(sharded_body, keep_unused=True)
    return fn, mesh, in_names, out_names, zero_outs


def _get_exec():
    global _EXEC
    if _EXEC is None:
        _EXEC = _build_exec()
    return _EXEC


def _concat_inputs(in_maps):
    """Per-core input dicts -> global concat arrays in executable order."""
    fn, mesh, in_names, out_names, zero_outs = _get_exec()
    concat_in = [
        np.concatenate([in_maps[c][name] for c in range(B)], axis=0)
        for name in in_names
    ]
    concat_zeros = [
        np.zeros((B * z.shape[0], *z.shape[1:]), z.dtype) for z in zero_outs
    ]
    return concat_in + concat_zeros


def kernel(
    inputs_for_keys: np.ndarray,
    inputs_for_values: np.ndarray,
    inputs_for_queries: np.ndarray,
    W_K: np.ndarray,
    W_V: np.ndarray,
    W_Q: np.ndarray,
) -> np.ndarray:
    fn, mesh, in_names, out_names, zero_outs = _get_exec()
    wk = np.ascontiguousarray(W_K, dtype=np.float32)
    wv = np.ascontiguousarray(W_V, dtype=np.float32)
    wq = np.ascontiguousarray(W_Q, dtype=np.float32)
    in_maps = [
        {
            "xk": np.ascontiguousarray(inputs_for_keys[b], dtype=np.float32),
            "xv": np.ascontiguousarray(inputs_for_values[b], dtype=np.float32),
            "xq": np.ascontiguousarray(inputs_for_queries[b], dtype=np.float32),
            "wk": wk,
            "wv": wv,
            "wq": wq,
        }
        for b in range(B)
    ]
    out_arrs = fn(*_concat_inputs(in_maps))
    z_all = np.asarray(out_arrs[out_names.index("z")])
    return z_all.reshape(B, S, D)


if __name__ == "__main__":
    rng = np.random.default_rng(0)
    ins = {
        "inputs_for_keys": rng.standard_normal((B, S, D), dtype=np.float32),
        "inputs_for_values": rng.standard_normal((B, S, D), dtype=np.float32),
        "inputs_for_queries": rng.standard_normal((B, S, D), dtype=np.float32),
        "W_K": (rng.standard_normal((D, D)) * 0.05).astype(np.float32),
        "W_V": (rng.standard_normal((D, D)) * 0.05).astype(np.float32),
        "W_Q": (rng.standard_normal((D, D)) * 0.05).astype(np.float32),
    }
    out = kernel(**ins)
    print("out", out.shape, out.dtype)


# revision 30
# speedup vs baseline: 1.7670x; 1.7670x over previous
"""Single-head attention (B=8, S=2048, D_in=D_out=1024) on 8 Trainium2 NeuronCores.

Sharding: data-parallel over batch — core b computes batch element b end-to-end.
Weights (W_K/W_V/W_Q, 4 MB each) are replicated to every core.

Design (vs the float32r baseline at ~500-580us measured): all matmul operands
are fp16 (same PE rate as float32r — 1 cyc/row — at half the SBUF footprint),
everything stays SBUF-resident, and the PE does no transposes at all:

  Projections. X [s,d] is DMA'd in fp32, cast to fp16 on the otherwise-idle
  GPSIMD engine, and transposed SBUF->SBUF per 128-row tile by the DMA XBAR
  ucode transpose (dma_start_transpose, 2-byte dtypes, 14ns per 16x128 tile)
  — the 384 PE identity-matmul transposes of the baseline become DMA-engine
  work that overlaps with PE matmuls.
    K^T tile [128 e, s]  = accum_d  W[d,e-slice]^T' @ X^T[d, s-chunk]
    Q^T tile [128 e, i]  = same (per query-half)
    V  tile [128 s, e]   = accum_d  X^T[d,s-slice]^T' @ W[d, e-chunk]
  Attention, per query-half (so Q^T 16KB + P^T 32KB coexist with K^T+V 64KB
  and the staging pools):
  B1 (scores, per 128-key tile j): computed directly TRANSPOSED:
    S^T chunk [128 j, 512 i] = accum_e kt[e][:, j-slice]^T' @ qt[e][:, i-chunk]
    P^T = exp(S^T/32 - 12) on ACT (PSUM fp32 in, fp16 SBUF out). The -12
    shift keeps P inside fp16 range (scores are O(+-13) for this data);
    softmax is shift-invariant so the 1/rowsum normalization cancels it.
    Scores come out already transposed, so the baseline's 256 PE transposes
    of P vanish and P^T feeds B2 directly as the stationary operand.
  B2 (output, per 128-query tile i):
    Z [128 i, e-512]  = accum_j pt[j][:, i-slice]^T' @ vt[j][:, e-chunk]
    rowsum [128 i, 1] = accum_j pt[j][:, i-slice]^T' @ ones[128,1]
      (reuses the stationary tile the PE just loaded for the Z matmuls —
      one extra moving row, nearly free)
    z = Z * (1/rowsum) fused into the PSUM->SBUF copy (DVE), DMA out fp32.

Scheduling (found by timeline-sim gap analysis):
  - Pipeline order per repeat: K-proj, Q-h0-proj, B1-h0, V-proj, B2-h0,
    Q-h1-proj, B1-h1, B2-h1. Each stage's DMA+cast demand lands inside the
    previous stage's PE window, so the serialized DMA resource is never
    oversubscribed against the PE (phase-A-only demand ~157us exceeds the
    A window; interleaved it doesn't).
  - All input DMAs and XBAR transposes issue on the SP queue; exp and the
    z-output DMAs on the ACT queue; fp32->fp16 casts run on GPSIMD. Keeping
    casts off the ACT/DVE queues matters: the tile scheduler encodes PSUM
    slot-reuse deps as engine-stream position thresholds, so a DMA-blocked
    cast hoisted ahead of the exps stalls B1's matmuls (cost 47us/rep).
  - All staging and resident pools are created once at top level and
    tag-rotated per repeat, so with R repeats in one NEFF, repeat n+1's
    X/W prefetch runs during repeat n's attention phases and the PE never
    waits on DMA at a repeat boundary.

PE budget per core: 1792 N=512 matmuls @ ~213ns + 256 N=1 matmuls ~= 405us
of PE busy (vs ~462us for the baseline, which adds 640 PE transposes at
fp32 2cyc/row). Timeline-sim steady-state slope 410us/rep (PE 98.5% busy);
measured harness slope ~300-310us/rep (vs 502-581us baseline).

Numerics: fp16 has a 10-bit mantissa; PSUM accumulation is fp32. Measured
end-to-end relative error vs the fp32 reference is 1.08e-3 (gate: 2e-2).
"""

from contextlib import ExitStack

import numpy as np

import concourse.bacc as bacc
import concourse.mybir as mybir
import concourse.tile as tile

F32 = mybir.dt.float32
H16 = mybir.dt.float16

B, S, D = 8, 2048, 1024
P = 128                    # SBUF partitions
TS = S // P                # 16 seq tiles
TD = D // P                # 8 d/e blocks
CH = 512                   # phase-A seq quarter (matmul free dim)
NCH = S // CH              # 4 quarters
TPC = CH // P              # 4 seq tiles per quarter
HS = S // 2                # query-half size for phase B
IC = 512                   # phase-B1 query chunk (mov free dim)
NIC = HS // IC             # 2 chunks per half
EC = 512                   # phase-B2 value-dim chunk
NEC = D // EC              # 2
SCALE = 1.0 / float(np.sqrt(D))
EXP_BIAS = -12.0           # softmax shift (cancelled by the 1/rowsum scale)


def build_program(repeats: int = 1, phases: str = "ab"):
    nc = bacc.Bacc("TRN2", target_bir_lowering=False, debug=False)

    xk = nc.dram_tensor("xk", [S, D], F32, kind="ExternalInput").ap()
    xv = nc.dram_tensor("xv", [S, D], F32, kind="ExternalInput").ap()
    xq = nc.dram_tensor("xq", [S, D], F32, kind="ExternalInput").ap()
    wk = nc.dram_tensor("wk", [D, D], F32, kind="ExternalInput").ap()
    wv = nc.dram_tensor("wv", [D, D], F32, kind="ExternalInput").ap()
    wq = nc.dram_tensor("wq", [D, D], F32, kind="ExternalInput").ap()
    z = nc.dram_tensor("z", [S, D], F32, kind="ExternalOutput").ap()

    with tile.TileContext(nc) as tc, ExitStack() as ctx:
        top = ctx.enter_context(tc.tile_pool(name="top", bufs=1))
        ones = top.tile([P, 1], H16, tag="ones", name="ones")
        nc.vector.memset(ones[:], 1.0)
        ebias = top.tile([P, 1], F32, tag="ebias", name="ebias")
        nc.vector.memset(ebias[:], EXP_BIAS)

        # Persistent pools: same tags rotate across repeats, which both
        # bounds SBUF and lets repeat n+1's staging DMAs overlap repeat n's
        # phase B (no address aliasing against the B-phase pools).
        pools = {
            "res": ctx.enter_context(tc.tile_pool(name="res", bufs=1)),
            "wst": ctx.enter_context(tc.tile_pool(name="wst", bufs=2)),
            "xst": ctx.enter_context(tc.tile_pool(name="xst", bufs=1)),
            "ptp": ctx.enter_context(tc.tile_pool(name="ptp", bufs=1)),
            "zop": ctx.enter_context(tc.tile_pool(name="zop", bufs=1)),
            "scp": ctx.enter_context(tc.tile_pool(name="scp", bufs=2)),
        }

        for rep in range(repeats):
            _one_pass(nc, tc, pools, ones, ebias, xk, xv, xq, wk, wv, wq, z, rep, phases)

    nc.compile()
    return nc


def _one_pass(nc, tc, pools, ones, ebias, xk, xv, xq, wk, wv, wq, z, rep, phases="ab"):
    res, wst, xst = pools["res"], pools["wst"], pools["xst"]
    ptp, zop, scp = pools["ptp"], pools["zop"], pools["scp"]

    # fp16 residents: K^T and Q^T as 8 [128 e, 2048 s] tiles, V as 16
    # [128 s, 1024 e] tiles. 96 KB/partition total.
    kt = [res.tile([P, S], H16, tag=f"kt{e}", name=f"kt{e}") for e in range(TD)]
    vt = [res.tile([P, D], H16, tag=f"v{j}", name=f"v{j}") for j in range(TS)]

    # ---------------- Phase A + B, Q interleaved per half ----------------
    with tc.tile_pool(name=f"psA{rep}", bufs=3, space="PSUM") as psA:

        def stage_x_quarter(x_dram, q):
            """Load+cast+XBAR-transpose one 512-row quarter of X into a
            [128 d_lo, 8 d_hi, 512 s] fp16 tile (DMA+Pool engines only)."""
            xtq = xst.tile([P, TD, CH], H16, tag="xtq", name="xtq", bufs=4)
            for t in range(TPC):
                row = (q * TPC + t) * P
                xbf = xst.tile([P, D], H16, tag="xbf", name="xbf", bufs=3)
                for xh in range(2):
                    xf = xst.tile([P, D // 2], F32, tag="xf", name="xf", bufs=4)
                    nc.sync.dma_start(
                        xf[:],
                        x_dram[row : row + P, xh * (D // 2) : (xh + 1) * (D // 2)],
                    )
                    nc.gpsimd.tensor_copy(
                        xbf[:, xh * (D // 2) : (xh + 1) * (D // 2)], xf[:]
                    )
                nc.sync.dma_start_transpose(xtq[:, :, t * P : (t + 1) * P], xbf[:])
            return xtq

        def stage_w(w_dram):
            """Load W fp32 and cast to fp16 [128 d_lo, 8 d_hi, 1024 e]."""
            wbf = wst.tile([P, TD, D], H16, tag="wbf", name="wbf")
            for dh in range(TD):
                for wh in range(2):
                    wf = wst.tile([P, D // 2], F32, tag="wf", name="wf", bufs=2)
                    nc.sync.dma_start(
                        wf[:],
                        w_dram[
                            dh * P : (dh + 1) * P,
                            wh * (D // 2) : (wh + 1) * (D // 2),
                        ],
                    )
                    nc.gpsimd.tensor_copy(
                        wbf[:, dh, wh * (D // 2) : (wh + 1) * (D // 2)], wf[:]
                    )
            return wbf

        def proj_kt(wbf, xtq, dst, q):
            # out^T tile [128 e, 512 s] = accum_d W[d,e-slice]^T' @ X^T[d,s]
            for e in range(TD):
                ps = psA.tile([P, CH], F32, tag="proj", name="proj_ps")
                for dh in range(TD):
                    nc.tensor.matmul(
                        ps[:],
                        wbf[:, dh, e * P : (e + 1) * P],
                        xtq[:, dh, :],
                        start=(dh == 0),
                        stop=(dh == TD - 1),
                    )
                nc.vector.tensor_copy(dst[e][:, q * CH : (q + 1) * CH], ps[:])

        def proj_v(wbf, xtq, q):
            # V tiles [128 s, 512 e] = accum_d X^T[d,s-slice]^T' @ W[d,e]
            for t in range(TPC):
                for ec in range(NEC):
                    ps = psA.tile([P, EC], F32, tag="proj", name="proj_ps")
                    for dh in range(TD):
                        nc.tensor.matmul(
                            ps[:],
                            xtq[:, dh, t * P : (t + 1) * P],
                            wbf[:, dh, ec * EC : (ec + 1) * EC],
                            start=(dh == 0),
                            stop=(dh == TD - 1),
                        )
                    nc.vector.tensor_copy(
                        vt[q * TPC + t][:, ec * EC : (ec + 1) * EC], ps[:]
                    )

        def proj_q_half(wbf_q, half, qt):
            for ql in range(NCH // 2):
                q = half * (NCH // 2) + ql
                xtq = stage_x_quarter(xq, q)
                for e in range(TD):
                    ps = psA.tile([P, CH], F32, tag="proj", name="proj_ps")
                    for dh in range(TD):
                        nc.tensor.matmul(
                            ps[:],
                            wbf_q[:, dh, e * P : (e + 1) * P],
                            xtq[:, dh, :],
                            start=(dh == 0),
                            stop=(dh == TD - 1),
                        )
                    nc.vector.tensor_copy(
                        qt[e][:, ql * CH : (ql + 1) * CH], ps[:]
                    )

        def b1_half(qt, pt):
            with tc.tile_pool(name=f"ps1_{rep}_{id(pt)}", bufs=2, space="PSUM") as ps1:
                for j in range(TS):
                    st = [
                        ps1.tile([P, IC], F32, tag=f"st{ic}", name=f"st{ic}")
                        for ic in range(NIC)
                    ]
                    for e in range(TD):
                        stat = kt[e][:, j * P : (j + 1) * P]
                        for ic in range(NIC):
                            nc.tensor.matmul(
                                st[ic][:],
                                stat,
                                qt[e][:, ic * IC : (ic + 1) * IC],
                                start=(e == 0),
                                stop=(e == TD - 1),
                            )
                    for ic in range(NIC):
                        # exp(s/32 - 12): the shift keeps P inside fp16
                        # range; the 1/rowsum normalization cancels it.
                        nc.scalar.activation(
                            pt[j][:, ic * IC : (ic + 1) * IC],
                            st[ic][:],
                            mybir.ActivationFunctionType.Exp,
                            scale=SCALE,
                            bias=ebias[:],
                        )

        def b2_half(half, pt):
            with tc.tile_pool(name=f"ps2_{rep}_{half}", bufs=2, space="PSUM") as ps2:
                for il in range(TS // 2):
                    it = half * (TS // 2) + il
                    zps = [
                        ps2.tile([P, EC], F32, tag=f"z{ec}", name=f"z{ec}")
                        for ec in range(NEC)
                    ]
                    sps = ps2.tile([P, 1], F32, tag="sm", name="sm", bufs=1)
                    for j in range(TS):
                        stat = pt[j][:, il * P : (il + 1) * P]
                        for ec in range(NEC):
                            nc.tensor.matmul(
                                zps[ec][:],
                                stat,
                                vt[j][:, ec * EC : (ec + 1) * EC],
                                start=(j == 0),
                                stop=(j == TS - 1),
                            )
                        nc.tensor.matmul(
                            sps[:],
                            stat,
                            ones[:],
                            start=(j == 0),
                            stop=(j == TS - 1),
                        )
                    rec = scp.tile([P, 1], F32, tag="rec", name="rec")
                    nc.vector.reciprocal(rec[:], sps[:])
                    for ec in range(NEC):
                        zo = zop.tile(
                            [P, EC], F32, tag=f"zo{ec}", name=f"zo{ec}", bufs=3
                        )
                        nc.vector.tensor_scalar_mul(zo[:], zps[ec][:], rec[:])
                        nc.scalar.dma_start(
                            z[it * P : (it + 1) * P, ec * EC : (ec + 1) * EC],
                            zo[:],
                        )

        # Pipeline order: K, Q-h0, B1-h0, V, B2-h0, Q-h1, B1-h1, B2-h1.
        # V's and Q-h1's DMA/cast demand lands inside the preceding
        # attention phases' compute windows, so the serialized DMA resource
        # is never oversubscribed against the PE.
        wbf_k = stage_w(wk)
        for q in range(NCH):
            proj_kt(wbf_k, stage_x_quarter(xk, q), kt, q)

        if phases == "a":
            # ablation: project V too, dump it as z.
            wbf_v = stage_w(wv)
            for q in range(NCH):
                proj_v(wbf_v, stage_x_quarter(xv, q), q)
            for it in range(TS):
                dummy = zop.tile([P, D], F32, tag="dummy", name="dummy")
                nc.vector.tensor_copy(dummy[:], vt[it][:])
                nc.scalar.dma_start(z[it * P : (it + 1) * P, :], dummy[:])
            return

        wbf_q = stage_w(wq)
        qt0 = [res.tile([P, HS], H16, tag=f"qt{e}", name=f"qt{e}") for e in range(TD)]
        proj_q_half(wbf_q, 0, qt0)
        pt0 = [ptp.tile([P, HS], H16, tag=f"pt{j}", name=f"pt{j}") for j in range(TS)]
        b1_half(qt0, pt0)

        wbf_v = stage_w(wv)
        for q in range(NCH):
            proj_v(wbf_v, stage_x_quarter(xv, q), q)

        b2_half(0, pt0)

        qt1 = [res.tile([P, HS], H16, tag=f"qt{e}", name=f"qt{e}") for e in range(TD)]
        proj_q_half(wbf_q, 1, qt1)
        pt1 = [ptp.tile([P, HS], H16, tag=f"pt{j}", name=f"pt{j}") for j in range(TS)]
        b1_half(qt1, pt1)
        b2_half(1, pt1)


_EXEC = None
_EXEC_BODY = None


def _build_exec(nc=None):
    """Compile the per-core program and wrap it in one jitted 8-core SPMD
    callable (shard_map over the 8 NeuronCores). Built once per process; the
    same callable serves correctness runs and timing loops."""
    import jax
    from jax.experimental.shard_map import shard_map
    from jax.sharding import Mesh, PartitionSpec

    from concourse import bass2jax

    if nc is None:
        nc = build_program()
    bass2jax.install_neuronx_cc_hook()

    partition_name = nc.partition_id_tensor.name if nc.partition_id_tensor else None
    in_names, out_names, out_avals, zero_outs = [], [], [], []
    for alloc in nc.m.functions[0].allocations:
        if not isinstance(alloc, mybir.MemoryLocationSet):
            continue
        name = alloc.memorylocations[0].name
        if alloc.kind == "ExternalInput":
            if name != partition_name:
                in_names.append(name)
        elif alloc.kind == "ExternalOutput":
            assert alloc.tensor_shape is not None and alloc.dtype is not None
            out_names.append(name)
            shape = tuple(alloc.tensor_shape)
            dtype = mybir.dt.np(alloc.dtype)
            out_avals.append(jax.core.ShapedArray(shape, dtype))
            zero_outs.append(np.zeros(shape, dtype))
    n_params = len(in_names)
    all_in_names = tuple(in_names) + tuple(out_names)
    if partition_name is not None:
        all_in_names = all_in_names + (partition_name,)

    def _body(*args):
        operands = list(args)
        if partition_name is not None:
            operands.append(bass2jax.partition_id_tensor())
        outs = bass2jax._bass_exec_p.bind(
            *operands,
            out_avals=tuple(out_avals),
            in_names=all_in_names,
            out_names=tuple(out_names),
            lowering_input_output_aliases=(),
            sim_require_finite=True,
            sim_require_nnan=True,
            nc=nc,
        )
        return tuple(outs)

    devices = jax.devices()[:B]
    assert len(devices) == B, f"need {B} cores, have {len(jax.devices())}"
    mesh = Mesh(np.asarray(devices), ("core",))
    n_outs = len(out_names)
    sharded_body = shard_map(
        _body,
        mesh=mesh,
        in_specs=(PartitionSpec("core"),) * (n_params + n_outs),
        out_specs=(PartitionSpec("core"),) * n_outs,
        check_rep=False,
    )
    global _EXEC_BODY
    _EXEC_BODY = sharded_body
    fn = jax.jit(sharded_body, keep_unused=True)
    return fn, mesh, in_names, out_names, zero_outs


def _get_exec():
    global _EXEC
    if _EXEC is None:
        _EXEC = _build_exec()
    return _EXEC


def _concat_inputs(in_maps):
    """Per-core input dicts -> global concat arrays in executable order."""
    fn, mesh, in_names, out_names, zero_outs = _get_exec()
    concat_in = [
        np.concatenate([in_maps[c][name] for c in range(B)], axis=0)
        for name in in_names
    ]
    concat_zeros = [
        np.zeros((B * z.shape[0], *z.shape[1:]), z.dtype) for z in zero_outs
    ]
    return concat_in + concat_zeros


def kernel(
    inputs_for_keys: np.ndarray,
    inputs_for_values: np.ndarray,
    inputs_for_queries: np.ndarray,
    W_K: np.ndarray,
    W_V: np.ndarray,
    W_Q: np.ndarray,
) -> np.ndarray:
    fn, mesh, in_names, out_names, zero_outs = _get_exec()
    wk = np.ascontiguousarray(W_K, dtype=np.float32)
    wv = np.ascontiguousarray(W_V, dtype=np.float32)
    wq = np.ascontiguousarray(W_Q, dtype=np.float32)
    in_maps = [
        {
            "xk": np.ascontiguousarray(inputs_for_keys[b], dtype=np.float32),
            "xv": np.ascontiguousarray(inputs_for_values[b], dtype=np.float32),
            "xq": np.ascontiguousarray(inputs_for_queries[b], dtype=np.float32),
            "wk": wk,
            "wv": wv,
            "wq": wq,
        }
        for b in range(B)
    ]
    out_arrs = fn(*_concat_inputs(in_maps))
    z_all = np.asarray(out_arrs[out_names.index("z")])
    return z_all.reshape(B, S, D)


if __name__ == "__main__":
    rng = np.random.default_rng(0)
    ins = {
        "inputs_for_keys": rng.standard_normal((B, S, D), dtype=np.float32),
        "inputs_for_values": rng.standard_normal((B, S, D), dtype=np.float32),
        "inputs_for_queries": rng.standard_normal((B, S, D), dtype=np.float32),
        "W_K": (rng.standard_normal((D, D)) * 0.05).astype(np.float32),
        "W_V": (rng.standard_normal((D, D)) * 0.05).astype(np.float32),
        "W_Q": (rng.standard_normal((D, D)) * 0.05).astype(np.float32),
    }
    out = kernel(**ins)
    print("out", out.shape, out.dtype)
